# revision 1
# baseline (speedup 1.0000x reference)
"""FlowNetC-style correlation (max_displacement=20, stride2=2, K=1) on 8 trn2 cores.

Math: out[b, ij, y, x] = (scale1*scale2/(96*out_scale)) *
        sum_c data1[b,c,y,x] * data2zp[b,c, y+dy, x+dx]
with ij = i*21 + j, dy = 2i-20, dx = 2j-20 and data2 zero-padded (pad cancels
against the output crop, so padding never materializes).

Strategy (per core = one batch element):
  - x is split by parity (dx is even so x and x+dx share parity): x = 2q+r;
    y likewise splits by parity yl (dy is even), y = 2*yh + yl.
  - Two y-rows (y0, y0+2) share one stationary operand (M=96 = 2x48 data1
    columns); the moving operand is the union of their 21-row data2 windows
    (22 rows), streamed once -- halving TensorE streaming vs per-row matmuls.
    PSUM tile [96, 22 slots x 48]: partition m = 48*g+q holds row y0+2g, slot
    s covers dy-index d0 = s-g.  The needed correlations are the 21 diagonals
    q' = q + dd of each [48,48] block.
  - PSUM tiles are copied to SBUF (DVE/ACT alternating), DMA'd padded to a
    DRAM scratch output, and the diagonals are gathered host-side with stride
    tricks (a per-partition-offset shear is not expressible on any engine AP).
  - scale factor is folded into data1 on the host; invalid (y,dy) tiles are
    never written and read back as zeros (outputs are zero-initialized).
"""

import os

import numpy as np

import concourse.bacc as bacc
import concourse.bass as bass
import concourse.mybir as mybir
import concourse.tile as tile
from concourse.bass_utils import run_bass_kernel_spmd

B, C, H, W = 8, 96, 64, 96
D = 21            # 21 displacements per axis (dy = 2*d0 - 20)
YH = H // 2       # 32  (y = 2*yh + yl)
Q = W // 2        # 48  (x = 2*q + r)
NSLOT = D + 1     # 22 dy-slots per row-pair (slot s -> d0 = s - g)
SLOTS_PER_BANK = 10   # 10 slots * 48 = 480 <= 512 fp32 per PSUM bank
BANK_F = 512
NBANKS = 3            # slots [0-9], [10-19], [20-21]
STAGE_F = NSLOT * Q   # 1056

# float32r streams 1 row/cycle on the PE (vs 4 for float32's LOW_HIGH pairs)
# and measures ~117us vs fp32's ~134us here, but carries ~1.7e-4 max-rel error
# (TF32-like reduced-precision products).  Default to exact-fp32 numerics.
COMPUTE_DT = os.environ.get("CORR_DT", "fp32")

_NC = None
LAST_RESULT = None


def slot_range(yh):
    """Valid slots s for row-pair starting at yh (yyh = yh-10+s in [0,32))."""
    return max(0, 10 - yh), min(NSLOT - 1, 31 + 10 - yh)


def _chunks(slo, shi):
    out = []
    for k in range(NBANKS):
        a = max(slo, k * SLOTS_PER_BANK)
        b = min(shi, min((k + 1) * SLOTS_PER_BANK, NSLOT) - 1)
        if a <= b:
            out.append((k, a, b))
    return out


def build_nc(compute_dt=None):
    compute_dt = compute_dt or COMPUTE_DT
    cdt = mybir.dt.float32r if compute_dt == "fp32r" else mybir.dt.float32
    nc = bacc.Bacc("TRN2", target_bir_lowering=False, debug=False, num_devices=B)
    f32 = mybir.dt.float32
    d1 = nc.dram_tensor("d1", [C, 2, 2, YH, Q], cdt, kind="ExternalInput")
    d2 = nc.dram_tensor("d2", [C, 2, 2, YH, Q], cdt, kind="ExternalInput")
    out = nc.dram_tensor(
        "out", [2, 2, YH // 2, 2 * Q, STAGE_F], f32, kind="ExternalOutput"
    )

    with tile.TileContext(nc) as tc:
        with (
            tc.tile_pool(name="inp", bufs=1) as inp,
            tc.tile_pool(name="psum", bufs=2, space=bass.MemorySpace.PSUM) as pp,
            tc.tile_pool(name="stage", bufs=8) as sp,
        ):
            s1 = inp.tile([C, 2, 2, YH, Q], cdt, tag="s1")
            s2 = inp.tile([C, 2, 2, YH, Q], cdt, tag="s2")
            # per-(yl,r) pieces, s2 in yh-halves, so the first units' matmuls
            # start as soon as their slice lands instead of after the full load
            for yl in range(2):
                for r in range(2):
                    nc.sync.dma_start(s1[:, yl, r], d1[:, yl, r])
                    nc.sync.dma_start(
                        s2[:, yl, r, 0 : YH // 2], d2[:, yl, r, 0 : YH // 2]
                    )
                    nc.sync.dma_start(
                        s2[:, yl, r, YH // 2 :], d2[:, yl, r, YH // 2 :]
                    )

            unit = 0
            for yl in range(2):
                for r in range(2):
                    for yhp in range(YH // 2):
                        yh = 2 * yhp
                        slo, shi = slot_range(yh)
                        ns = shi - slo + 1
                        chunks = _chunks(slo, shi)

                        pt = pp.tile([2 * Q, NBANKS * BANK_F], mybir.dt.float32,
                                     tag="pt")
                        st = sp.tile([2 * Q, STAGE_F], mybir.dt.float32, tag="st")

                        lhsT = s1[:, yl, r, yh : yh + 2, :]
                        for k, a, b in chunks:
                            rhs = s2[:, yl, r, yh - 10 + a : yh - 10 + b + 1, :]
                            po = k * BANK_F + (a - k * SLOTS_PER_BANK) * Q
                            n = (b - a + 1) * Q
                            nc.tensor.matmul(
                                pt[:, po : po + n], lhsT, rhs,
                                start=True, stop=True,
                            )

                        # split each unit's copies across DVE and ACT so the
                        # PSUM slot frees fast and the PE never idles on it
                        dst0 = 0
                        for ci, (k, a, b) in enumerate(chunks):
                            po = k * BANK_F + (a - k * SLOTS_PER_BANK) * Q
                            n = (b - a + 1) * Q
                            if (ci + unit) % 2 == 0:
                                nc.vector.tensor_copy(
                                    st[:, dst0 : dst0 + n], pt[:, po : po + n]
                                )
                            else:
                                nc.scalar.copy(
                                    st[:, dst0 : dst0 + n], pt[:, po : po + n]
                                )
                            dst0 += n

                        nc.sync.dma_start(
                            out[yl, r, yhp, :, slo * Q : (shi + 1) * Q],
                            st[:, 0 : ns * Q],
                        )
                        unit += 1

    nc.compile()
    return nc


def _get_nc():
    global _NC
    if _NC is None:
        _NC = build_nc()
    return _NC


def _prep(x):
    """[C, H, W] f32 -> [C, 2(yl), 2(r), YH, Q] contiguous."""
    return np.ascontiguousarray(
        x.reshape(C, YH, 2, Q, 2).transpose(0, 2, 4, 1, 3)
    )


def assemble(scratch, out_b):
    """Gather the 21 banded diagonals of each all-pairs tile into out_b.

    scratch: [2, 2, YH//2, 96, STAGE_F] f32 (zeros where never written).
    out_b:   [D*D, H, W] f32, pre-zeroed.
    """
    scratch = np.ascontiguousarray(scratch)
    outv = out_b.reshape(D, D, H, W)
    s_hp, s_m, s_f = scratch.strides[2:]
    for yl in range(2):
        for r in range(2):
            for g in range(2):
                for dd in range(-10, 11):
                    q0 = max(0, -dd)
                    ln = Q - abs(dd)
                    base = scratch[yl, r, :, Q * g + q0 :, Q * g + q0 + dd :]
                    view = np.lib.stride_tricks.as_strided(
                        base,
                        shape=(YH // 2, D, ln),
                        strides=(s_hp, Q * s_f, s_m + s_f),
                    )
                    outv[
                        :, dd + 10, yl + 2 * g :: 4,
                        r + 2 * q0 : r + 2 * (q0 + ln) : 2,
                    ] = view.swapaxes(0, 1)


def kernel(data1, data2, scale1, scale2, inter_scale, out_scale):
    data1 = np.asarray(data1, np.float32)
    data2 = np.asarray(data2, np.float32)
    factor = (
        float(np.asarray(scale1).reshape(-1)[0])
        * float(np.asarray(scale2).reshape(-1)[0])
        / (float(C) * float(np.asarray(out_scale).reshape(-1)[0]))
    )
    d1s = data1 * np.float32(factor)

    in_maps = [
        {"d1": _prep(d1s[b]), "d2": _prep(data2[b])} for b in range(B)
    ]
    res = run_bass_kernel_spmd(_get_nc(), in_maps, list(range(B)))
    global LAST_RESULT
    LAST_RESULT = res

    out = np.zeros((B, D * D, H, W), np.float32)
    for b in range(B):
        assemble(res.results[b]["out"], out[b])
    return out

